# revision 12
# baseline (speedup 1.0000x reference)
"""AssignIndex (scatter) kernel for Trainium2, SPMD across 8 NeuronCores.

out = arr, except out[index, :] = element.

Strategy (per the sharding hint): shard arr row-wise across the 8 cores
(8192 rows x 1024 f32 = 32 MiB per core).  Every core runs the identical
SPMD graph: DMA-copy its shard DRAM->DRAM at the HBM roofline, except
the one local row that is written from a per-core "patch" input.  For
the core owning the global `index` row the patch equals `element`; for
every other core the patch equals that core's own original row at the
same local offset, so the write is a data no-op and a single SPMD graph
stays correct without any control-flow divergence.

Performance notes (measured on trn2 via neuron-profile):
- A single dma_start per core saturates one DMA queue at ~170 GB/s
  payload (~212 us).  Splitting the copy across the three DMA-issuing
  engines (sync + scalar = two HWDGE rings, gpsimd = SWDGE ring) lets
  the 16 SDMA engines interleave packets from three rings and roughly
  doubles throughput (~115 us, ~0.58 TB/s read+write per core, which is
  the practical HBM wall here; per-domain ~1.15 TB/s shared by the two
  cores of a pair).
- The HWDGE streams are each split into 2 chunks; the SWDGE stream is
  kept as 1 chunk (measured best: queue tails drain more evenly).
"""

import os
import sys

for _p in ("/opt/trn_rl_repo",):
    if _p not in sys.path and os.path.isdir(_p):
        sys.path.insert(0, _p)

import numpy as np

N_CORES = 8

# sync/scalar/gpsimd streams: fraction of rows and chunk count per stream.
_CHUNKS_PER_QUEUE = (2, 2, 1)

# Populated with the most recent BassKernelResults (exec_time_ns etc.)
LAST_RESULT = None


def _split_rows(segments, n_queues):
    """Cut contiguous row segments into n_queues ~equal-row groups."""
    total = sum(e - s for s, e in segments)
    cuts = [round(total * k / n_queues) for k in range(1, n_queues)]
    assignments = [[] for _ in range(n_queues)]
    qi, done = 0, 0
    for s, e in segments:
        pos = s
        while pos < e:
            limit = cuts[qi] if qi < len(cuts) else total
            take = min(e - pos, limit - done)
            if take > 0:
                assignments[qi].append((pos, pos + take))
                pos += take
                done += take
            if qi < len(cuts) and done >= cuts[qi]:
                qi += 1
    return assignments


def _build(rows_per_core, D, local_row, write_patch):
    import concourse.bass as bass
    import concourse.mybir as mybir

    nc = bass.Bass()
    arr = nc.declare_dram_parameter(
        "arr", [rows_per_core, D], mybir.dt.float32, isOutput=False
    )
    patch = nc.declare_dram_parameter(
        "patch", [1, D], mybir.dt.float32, isOutput=False
    )
    out = nc.declare_dram_parameter(
        "out", [rows_per_core, D], mybir.dt.float32, isOutput=True
    )

    segments = []
    if write_patch:
        if local_row > 0:
            segments.append((0, local_row))
        if local_row + 1 < rows_per_core:
            segments.append((local_row + 1, rows_per_core))
    else:
        segments.append((0, rows_per_core))

    assignments = _split_rows(segments, 3)
    for q, n_chunks in enumerate(_CHUNKS_PER_QUEUE):
        if n_chunks > 1:
            new_chunks = []
            for s, e in assignments[q]:
                step = max(1, (e - s + n_chunks - 1) // n_chunks)
                for p in range(s, e, step):
                    new_chunks.append((p, min(p + step, e)))
            assignments[q] = new_chunks

    with (
        nc.Block() as block,
        nc.semaphore("dma_sem") as dma_sem,
        nc.semaphore("dma_sem2") as dma_sem2,
        nc.semaphore("dma_sem3") as dma_sem3,
    ):
        # All copied regions are disjoint from the patched row, so the
        # three streams have no ordering constraints between them; each
        # engine only waits for its own DMA completions.

        @block.sync
        def _(sync):
            expected = 0
            for s, e in assignments[0]:
                sync.dma_start(out=out[s:e], in_=arr[s:e]).then_inc(dma_sem, 16)
                expected += 16
            if write_patch:
                sync.dma_start(
                    out=out[local_row : local_row + 1], in_=patch[:]
                ).then_inc(dma_sem, 16)
                expected += 16
            if expected:
                sync.wait_ge(dma_sem, expected)

        @block.scalar
        def _(scalar):
            expected = 0
            for s, e in assignments[1]:
                scalar.dma_start(out=out[s:e], in_=arr[s:e]).then_inc(
                    dma_sem2, 16
                )
                expected += 16
            if expected:
                scalar.wait_ge(dma_sem2, expected)

        @block.gpsimd
        def _(gpsimd):
            expected = 0
            for s, e in assignments[2]:
                gpsimd.dma_start(out=out[s:e], in_=arr[s:e]).then_inc(
                    dma_sem3, 16
                )
                expected += 16
            if expected:
                gpsimd.wait_ge(dma_sem3, expected)

    return nc


def kernel(arr, index, element):
    global LAST_RESULT
    from concourse.bass_utils import run_bass_kernel_spmd

    arr = np.ascontiguousarray(np.asarray(arr, dtype=np.float32))
    element = np.ascontiguousarray(
        np.asarray(element, dtype=np.float32)
    ).reshape(-1)
    N, D = arr.shape
    idx = int(index)
    rows = N // N_CORES
    assert rows * N_CORES == N

    # Out-of-range index: one_hot(index, N) is all-zero -> output == arr.
    write_patch = 0 <= idx < N
    if write_patch:
        owner, local = divmod(idx, rows)
    else:
        owner, local = -1, 0

    in_maps = []
    for c in range(N_CORES):
        shard = arr[c * rows : (c + 1) * rows]
        p = element if c == owner else shard[local]
        in_maps.append(
            {"arr": shard, "patch": np.ascontiguousarray(p.reshape(1, D))}
        )

    nc = _build(rows, D, local, write_patch)
    res = run_bass_kernel_spmd(nc, in_maps, core_ids=list(range(N_CORES)))
    LAST_RESULT = res
    return np.concatenate(
        [res.results[c]["out"] for c in range(N_CORES)], axis=0
    )
